# revision 3
# baseline (speedup 1.0000x reference)
# Trainium2 Bass kernel for nn_CoordinateDecoder (self-contained).
#
# Strategy (per core = one batch element, data-parallel over B=8):
#  - Host: sort points by coords[:,0] (one permutation makes the bilinear
#    y-row groups contiguous at all 3 pyramid levels), build per-level 2-nnz
#    x-interpolation matrices (with the y-row weights folded in), positional
#    encoding, and fold the FiLM gamma/beta into per-batch MLP weights.
#  - Device: build the 2-level downsampled pyramid with PE matmuls
#    (y-contraction with per-d stationary grid planes, then x-contraction),
#    sample all 3 levels with grouped matmuls (s_g^T = G[r]^T @ U_g summed
#    over the two rows in PSUM), then run the FiLM-folded MLP in transposed
#    activation layout [features, tokens] with bias+gelu fused into the
#    PSUM->SBUF evacuation on the scalar engine.
import sys

if "/opt/trn_rl_repo" not in sys.path:
    sys.path.insert(0, "/opt/trn_rl_repo")

import numpy as np

import concourse.bass as bass
import concourse.mybir as mybir
import concourse.tile as tile
from concourse import bacc
from concourse.bass_utils import run_bass_kernel_spmd

B, H, W, D = 8, 128, 128, 256
N = 8192
NUM_FREQS = 10
MLP_W = 256
DEPTH = 4
NCORES = 8
F16 = mybir.dt.float16
F32 = mybir.dt.float32
GELU = mybir.ActivationFunctionType.Gelu_apprx_tanh
TANH = mybir.ActivationFunctionType.Tanh
TOK_CHUNK = 1024


# ----------------------------------------------------------------- host math
def _resize_matrix(in_size: int, out_size: int) -> np.ndarray:
    # port of jax.image.resize(..., 'bilinear', antialias=True) weights
    scale = out_size / in_size
    sample_f = (np.arange(out_size, dtype=np.float64) + 0.5) / scale - 0.5
    x = np.abs(sample_f[None, :] - np.arange(in_size, dtype=np.float64)[:, None]) * scale
    weights = np.maximum(0.0, 1.0 - x)
    total = weights.sum(axis=0, keepdims=True)
    weights = np.where(np.abs(total) > 1000.0 * np.finfo(np.float32).eps, weights / total, 0.0)
    weights = np.where(
        np.logical_and(sample_f[None, :] >= -0.5, sample_f[None, :] <= in_size - 0.5),
        weights, 0.0)
    return weights.T.astype(np.float32)  # [out, in]


def _positional_encoding(coords: np.ndarray) -> np.ndarray:
    freqs = (2.0 ** np.arange(NUM_FREQS, dtype=np.float32)) * np.float32(np.pi)
    ang = coords[:, None, :] * freqs[None, :, None]
    sc = np.stack([np.sin(ang), np.cos(ang)], axis=2)
    return np.concatenate([coords, sc.reshape(coords.shape[0], -1)], axis=-1).astype(np.float32)


def _sample_prep(coords: np.ndarray, Hl: int, Wl: int):
    n = coords.shape[0]
    y = (coords[:, 0].astype(np.float64) + 1.0) * 0.5 * (Hl - 1)
    x = (coords[:, 1].astype(np.float64) + 1.0) * 0.5 * (Wl - 1)
    y0f = np.clip(np.floor(y), 0.0, Hl - 1)
    x0f = np.clip(np.floor(x), 0.0, Wl - 1)
    y0 = y0f.astype(np.int64)
    x0 = x0f.astype(np.int64)
    wy = (y - y0f).astype(np.float32)
    wx = (x - x0f).astype(np.float32)
    at_edge_y = y0 >= Hl - 1
    y0_eff = np.where(at_edge_y, Hl - 2, y0)
    wa = np.where(at_edge_y, 0.0, 1.0 - wy).astype(np.float32)
    wb = np.where(at_edge_y, 1.0, wy).astype(np.float32)
    at_edge_x = x0 >= Wl - 1
    x0_eff = np.where(at_edge_x, Wl - 2, x0)
    ux0 = np.where(at_edge_x, 0.0, 1.0 - wx).astype(np.float32)
    ux1 = np.where(at_edge_x, 1.0, wx).astype(np.float32)
    u = np.zeros((Wl, n), dtype=np.float32)
    cols = np.arange(n)
    u[x0_eff, cols] = ux0
    u[x0_eff + 1, cols] = ux1
    return y0_eff, u * wa[None, :], u * wb[None, :]


def _host_prep(inputs: dict):
    coords = np.asarray(inputs["coords"], np.float32)
    context = np.asarray(inputs["context_vector"], np.float32)
    ctx_w = np.asarray(inputs["ctx_w"], np.float32)
    ctx_b = np.asarray(inputs["ctx_b"], np.float32)
    mlp0_w = np.asarray(inputs["mlp0_w"], np.float32)
    mlp0_b = np.asarray(inputs["mlp0_b"], np.float32)
    mlp_hw = np.asarray(inputs["mlp_hw"], np.float32)
    mlp_hb = np.asarray(inputs["mlp_hb"], np.float32)
    out_w = np.asarray(inputs["out_w"], np.float32)
    out_b = np.asarray(inputs["out_b"], np.float32)
    oracle = np.asarray(inputs["oracle_pixels"], np.float32)

    perm = np.argsort(coords[:, 0], kind="stable")
    cs = coords[perm]
    enc = _positional_encoding(cs)  # [N, 42]

    offs = []
    us = []
    for Hl, Wl in [(H, W), (H // 2, W // 2), (H // 4, W // 4)]:
        y0, ua, ub = _sample_prep(cs, Hl, Wl)
        offs.append(np.searchsorted(y0, np.arange(Hl + 1)).astype(np.int64))
        us.append((ua.astype(np.float16), ub.astype(np.float16)))

    ctx = context @ ctx_w + ctx_b
    gamma = ctx[:, :MLP_W] + 1.0
    beta = ctx[:, MLP_W:]

    shared = {
        "enc_t": enc.T.astype(np.float16).copy(),  # [42, N]
        "u0a": us[0][0], "u0b": us[0][1],          # [128, N]
        "u1a": us[1][0], "u1b": us[1][1],          # [64, N]
        "u2a": us[2][0], "u2b": us[2][1],          # [32, N]
        "ah_stack": np.concatenate(
            [_resize_matrix(H, H // 2), _resize_matrix(H, H // 4)], axis=0
        ).T.astype(np.float16).copy(),             # [128(y), 96]
        "aw1t": _resize_matrix(W, W // 2).T.astype(np.float16).copy(),  # [128, 64]
        "aw2t": _resize_matrix(W, W // 4).T.astype(np.float16).copy(),  # [128, 32]
        "wout": out_w.astype(np.float16).copy(),   # [256, 3]
        "bout": out_b.reshape(3, 1).astype(np.float32).copy(),
    }
    per_core = []
    for b in range(B):
        w0 = (mlp0_w * gamma[b][None, :]).astype(np.float16)        # [813, 256]
        b0 = (mlp0_b * gamma[b] + beta[b]).astype(np.float32)       # [256]
        wh = (mlp_hw * gamma[b][None, None, :]).astype(np.float16)  # [3, 256, 256]
        bh = (mlp_hb * gamma[b][None, :] + beta[b][None, :]).astype(np.float32)
        per_core.append({
            "grid": np.asarray(inputs["feature_grid"][b], np.float32).reshape(H, W * D),
            "oracle_t": np.ascontiguousarray(oracle[b][perm].T).astype(np.float16),  # [3, N]
            "w0": w0,
            "wh": wh,
            "b0": np.ascontiguousarray(b0.reshape(2, 128).T),   # [128, 2]
            "bh": np.ascontiguousarray(bh.reshape(3, 2, 128).transpose(2, 0, 1).reshape(128, 6)),
            **shared,
        })
    return per_core, perm, offs


# ------------------------------------------------------------- device kernel
def _build_program(offs):
    nc = bacc.Bacc("TRN2", target_bir_lowering=False, debug=False, num_devices=NCORES)

    grid = nc.dram_tensor("grid", [H, W * D], F32, kind="ExternalInput")
    u0a = nc.dram_tensor("u0a", [128, N], F16, kind="ExternalInput")
    u0b = nc.dram_tensor("u0b", [128, N], F16, kind="ExternalInput")
    u1a = nc.dram_tensor("u1a", [64, N], F16, kind="ExternalInput")
    u1b = nc.dram_tensor("u1b", [64, N], F16, kind="ExternalInput")
    u2a = nc.dram_tensor("u2a", [32, N], F16, kind="ExternalInput")
    u2b = nc.dram_tensor("u2b", [32, N], F16, kind="ExternalInput")
    enc_t = nc.dram_tensor("enc_t", [42, N], F16, kind="ExternalInput")
    oracle_t = nc.dram_tensor("oracle_t", [3, N], F16, kind="ExternalInput")
    ah_stack = nc.dram_tensor("ah_stack", [128, 96], F16, kind="ExternalInput")
    aw1t = nc.dram_tensor("aw1t", [128, 64], F16, kind="ExternalInput")
    aw2t = nc.dram_tensor("aw2t", [128, 32], F16, kind="ExternalInput")
    w0 = nc.dram_tensor("w0", [813, 256], F16, kind="ExternalInput")
    wh = nc.dram_tensor("wh", [3, 256, 256], F16, kind="ExternalInput")
    wout = nc.dram_tensor("wout", [256, 3], F16, kind="ExternalInput")
    b0 = nc.dram_tensor("b0", [128, 2], F32, kind="ExternalInput")
    bh = nc.dram_tensor("bh", [128, 6], F32, kind="ExternalInput")
    bout = nc.dram_tensor("bout", [3, 1], F32, kind="ExternalInput")
    out_t = nc.dram_tensor("out_t", [3, N], F32, kind="ExternalOutput")

    o0, o1, o2 = offs

    with tile.TileContext(nc) as tc:
        # ---------------- persistent small tensors
        with tc.tile_pool(name="persist", bufs=1) as persist:
            ah_sb = persist.tile([128, 96], F16, tag="ah")
            nc.sync.dma_start(out=ah_sb, in_=ah_stack[:, :])
            aw1_sb = persist.tile([128, 64], F16, tag="aw1")
            nc.sync.dma_start(out=aw1_sb, in_=aw1t[:, :])
            aw2_sb = persist.tile([128, 32], F16, tag="aw2")
            nc.sync.dma_start(out=aw2_sb, in_=aw2t[:, :])
            b0_sb = persist.tile([128, 2], F32, tag="b0")
            nc.sync.dma_start(out=b0_sb, in_=b0[:, :])
            bh_sb = persist.tile([128, 6], F32, tag="bh")
            nc.sync.dma_start(out=bh_sb, in_=bh[:, :])
            bout_sb = persist.tile([3, 1], F32, tag="bout")
            nc.sync.dma_start(out=bout_sb, in_=bout[:, :])
            # MLP weights in SBUF: w0 split into 8 k-parts
            w0_parts = []
            w0_bounds = [0, 42, 170, 298, 426, 554, 682, 810, 813]
            for i in range(8):
                lo, hi = w0_bounds[i], w0_bounds[i + 1]
                t = persist.tile([hi - lo, 256], F16, tag=f"w0_{i}", name=f"w0_{i}")
                nc.sync.dma_start(out=t, in_=w0[lo:hi, :])
                w0_parts.append(t)
            wh_parts = []
            for i in range(DEPTH - 1):
                row = []
                for k in range(2):
                    t = persist.tile([128, 256], F16, tag=f"wh_{i}_{k}", name=f"wh_{i}_{k}")
                    nc.sync.dma_start(out=t, in_=wh[i, k * 128:(k + 1) * 128, :])
                    row.append(t)
                wh_parts.append(row)
            wout_parts = []
            for k in range(2):
                t = persist.tile([128, 3], F16, tag=f"wout_{k}", name=f"wout_{k}")
                nc.sync.dma_start(out=t, in_=wout[k * 128:(k + 1) * 128, :])
                wout_parts.append(t)

            g1_sb = persist.tile([64, 64 * 256], F16, tag="g1")
            g2_sb = persist.tile([32, 32 * 256], F16, tag="g2")

            with tc.tile_pool(name="psum", bufs=8, space="PSUM") as psum:
                # ---------------- pyramid
                with tc.tile_pool(name="pyr", bufs=1) as pyr, \
                     tc.tile_pool(name="stage", bufs=3) as stage:
                    g0y = pyr.tile([128, W * D], F16, tag="g0y")
                    n_ld = 32
                    ld = (W * D) // n_ld
                    for j in range(n_ld):
                        st = stage.tile([128, ld], F32, tag="st")
                        nc.sync.dma_start(out=st, in_=grid[:, j * ld:(j + 1) * ld])
                        if j % 2 == 0:
                            nc.vector.tensor_copy(out=g0y[:, j * ld:(j + 1) * ld], in_=st)
                        else:
                            nc.scalar.copy(out=g0y[:, j * ld:(j + 1) * ld], in_=st)

                    p2 = pyr.tile([128, 256 * 96], F16, tag="p2")
                    # stage 1: per-d y-contraction, 4 d per PSUM bank
                    for j in range(64):
                        ps = psum.tile([128, 384], F32, tag="ps")
                        for i in range(4):
                            d = 4 * j + i
                            nc.tensor.matmul(
                                out=ps[:, i * 96:(i + 1) * 96],
                                lhsT=g0y[:, d:W * D:256],
                                rhs=ah_sb,
                                start=True, stop=True,
                            )
                        if j % 2 == 0:
                            nc.vector.tensor_copy(out=p2[:, j * 384:(j + 1) * 384], in_=ps)
                        else:
                            nc.scalar.copy(out=p2[:, j * 384:(j + 1) * 384], in_=ps)

                    # stage 2: x-contraction -> G1 rows / G2 rows
                    for r in range(64):
                        ps = psum.tile([64, 256], F32, tag="ps")
                        nc.tensor.matmul(out=ps, lhsT=aw1_sb,
                                         rhs=p2[:, r:256 * 96:96],
                                         start=True, stop=True)
                        if r % 2 == 0:
                            nc.vector.tensor_copy(out=g1_sb[:, r * 256:(r + 1) * 256], in_=ps)
                        else:
                            nc.scalar.copy(out=g1_sb[:, r * 256:(r + 1) * 256], in_=ps)
                    for r in range(32):
                        ps = psum.tile([32, 256], F32, tag="ps")
                        nc.tensor.matmul(out=ps, lhsT=aw2_sb,
                                         rhs=p2[:, 64 + r:256 * 96:96],
                                         start=True, stop=True)
                        if r % 2 == 0:
                            nc.vector.tensor_copy(out=g2_sb[:, r * 256:(r + 1) * 256], in_=ps)
                        else:
                            nc.scalar.copy(out=g2_sb[:, r * 256:(r + 1) * 256], in_=ps)

                # ---------------- sampling + MLP, chunked over tokens
                with tc.tile_pool(name="rows", bufs=24) as rows, \
                     tc.tile_pool(name="rstage", bufs=4) as rstage, \
                     tc.tile_pool(name="uchunk", bufs=2) as uchunk, \
                     tc.tile_pool(name="schunk", bufs=2) as schunk, \
                     tc.tile_pool(name="hchunk", bufs=2) as hchunk, \
                     tc.tile_pool(name="ochunk", bufs=2) as ochunk:
                    n_chunks = N // TOK_CHUNK
                    for c in range(n_chunks):
                        t0, t1 = c * TOK_CHUNK, (c + 1) * TOK_CHUNK
                        # chunk-local streamed inputs
                        u_sb = {}
                        for name, t, parts in (("u0a", u0a, 128), ("u0b", u0b, 128),
                                               ("u1a", u1a, 64), ("u1b", u1b, 64),
                                               ("u2a", u2a, 32), ("u2b", u2b, 32)):
                            ut = uchunk.tile([parts, TOK_CHUNK], F16, tag=name,
                                             name=name)
                            nc.sync.dma_start(out=ut, in_=t[:, t0:t1])
                            u_sb[name] = ut
                        encc = uchunk.tile([42, TOK_CHUNK], F16, tag="encc")
                        nc.sync.dma_start(out=encc, in_=enc_t[:, t0:t1])
                        orcc = uchunk.tile([3, TOK_CHUNK], F16, tag="orcc")
                        nc.sync.dma_start(out=orcc, in_=oracle_t[:, t0:t1])

                        s_sb = {}  # (lvl, half) -> [128, TOK_CHUNK] f16
                        for lvl in range(3):
                            for hf in range(2):
                                s_sb[(lvl, hf)] = schunk.tile(
                                    [128, TOK_CHUNK], F16, tag=f"s{lvl}{hf}",
                                    name=f"s{lvl}{hf}")

                        # ---- level 0 sampling: stream grid rows x-oriented
                        def level_sample(lvl, offv, n_rows_l, ua, ub, row_of):
                            evac = 0
                            for r in range(n_rows_l - 1):
                                s0 = max(int(offv[r]), t0)
                                s1 = min(int(offv[r + 1]), t1)
                                if s1 <= s0:
                                    continue
                                la, lb = s0 - t0, s1 - t0
                                ra = row_of(r)
                                rb = row_of(r + 1)
                                for hf in range(2):
                                    ps = psum.tile([128, 512], F32, tag="ps")
                                    nc.tensor.matmul(
                                        out=ps[:, :lb - la],
                                        lhsT=ra[:, hf * 128:(hf + 1) * 128],
                                        rhs=ua[:, la:lb],
                                        start=True, stop=False)
                                    nc.tensor.matmul(
                                        out=ps[:, :lb - la],
                                        lhsT=rb[:, hf * 128:(hf + 1) * 128],
                                        rhs=ub[:, la:lb],
                                        start=False, stop=True)
                                    if evac % 2 == 0:
                                        nc.vector.tensor_copy(
                                            out=s_sb[(lvl, hf)][:, la:lb],
                                            in_=ps[:, :lb - la])
                                    else:
                                        nc.scalar.copy(
                                            out=s_sb[(lvl, hf)][:, la:lb],
                                            in_=ps[:, :lb - la])
                                    evac += 1

                        # level 0: rows from HBM
                        lo_row = int(np.searchsorted(o0[1:], t0, side="right"))
                        hi_row = int(np.searchsorted(o0[:-1], t1, side="left"))
                        hi_row = min(hi_row, H - 1)
                        row_tiles = {}
                        for r in range(lo_row, hi_row + 2):
                            if r > H - 1:
                                break
                            rf32 = rstage.tile([128, 256], F32, tag="rf32")
                            nc.sync.dma_start(
                                out=rf32,
                                in_=bass.AP(grid, r * W * D, [[256, 128], [1, 256]]))
                            rf16 = rows.tile([128, 256], F16, tag="rf16")
                            if r % 2 == 0:
                                nc.vector.tensor_copy(out=rf16, in_=rf32)
                            else:
                                nc.scalar.copy(out=rf16, in_=rf32)
                            row_tiles[r] = rf16

                        level_sample(0, o0, H, u_sb["u0a"], u_sb["u0b"],
                                     lambda r: row_tiles[r])
                        level_sample(1, o1, H // 2, u_sb["u1a"], u_sb["u1b"],
                                     lambda r: g1_sb[:, r * 256:(r + 1) * 256])
                        level_sample(2, o2, H // 4, u_sb["u2a"], u_sb["u2b"],
                                     lambda r: g2_sb[:, r * 256:(r + 1) * 256])

                        # ---- MLP on this chunk (transposed layout)
                        l1_rhs = [encc, s_sb[(0, 0)], s_sb[(0, 1)], s_sb[(1, 0)],
                                  s_sb[(1, 1)], s_sb[(2, 0)], s_sb[(2, 1)], orcc]
                        l1_lhs = w0_parts  # order: enc, s0h0, s0h1, ..., oracle
                        h_cur = [hchunk.tile([128, TOK_CHUNK], F16, tag=f"h{hf}",
                                             name=f"h{hf}")
                                 for hf in range(2)]
                        n_sub = TOK_CHUNK // 512
                        for hf in range(2):
                            for s in range(n_sub):
                                ps = psum.tile([128, 512], F32, tag="ps")
                                for k in range(8):
                                    nc.tensor.matmul(
                                        out=ps,
                                        lhsT=l1_lhs[k][:, hf * 128:(hf + 1) * 128],
                                        rhs=l1_rhs[k][:, s * 512:(s + 1) * 512],
                                        start=(k == 0), stop=(k == 7))
                                nc.scalar.activation(
                                    out=h_cur[hf][:, s * 512:(s + 1) * 512],
                                    in_=ps, func=GELU, bias=b0_sb[:, hf:hf + 1])
                        for layer in range(DEPTH - 1):
                            h_nxt = [hchunk.tile([128, TOK_CHUNK], F16,
                                                 tag=f"hn{layer % 2}{hf}",
                                                 name=f"hn{layer % 2}{hf}")
                                     for hf in range(2)]
                            for hf in range(2):
                                for s in range(n_sub):
                                    ps = psum.tile([128, 512], F32, tag="ps")
                                    for k in range(2):
                                        nc.tensor.matmul(
                                            out=ps,
                                            lhsT=wh_parts[layer][k][:, hf * 128:(hf + 1) * 128],
                                            rhs=h_cur[k][:, s * 512:(s + 1) * 512],
                                            start=(k == 0), stop=(k == 1))
                                    nc.scalar.activation(
                                        out=h_nxt[hf][:, s * 512:(s + 1) * 512],
                                        in_=ps, func=GELU,
                                        bias=bh_sb[:, layer * 2 + hf:layer * 2 + hf + 1])
                            h_cur = h_nxt
                        # output layer + tanh
                        oc = ochunk.tile([3, TOK_CHUNK], F32, tag="oc")
                        for s in range(n_sub):
                            ps = psum.tile([3, 512], F32, tag="ps")
                            for k in range(2):
                                nc.tensor.matmul(
                                    out=ps, lhsT=wout_parts[k],
                                    rhs=h_cur[k][:, s * 512:(s + 1) * 512],
                                    start=(k == 0), stop=(k == 1))
                            nc.scalar.activation(out=oc[:, s * 512:(s + 1) * 512],
                                                 in_=ps, func=TANH, bias=bout_sb)
                        nc.sync.dma_start(out=out_t[:, t0:t1], in_=oc)

    nc.compile()
    return nc


# ------------------------------------------------------------------ wrapper
_cache = {}


def kernel(**inputs) -> np.ndarray:
    per_core, perm, offs = _host_prep(inputs)
    key = tuple(tuple(int(v) for v in o) for o in offs)
    if key not in _cache:
        _cache.clear()
        _cache[key] = _build_program(offs)
    nc = _cache[key]
    res = run_bass_kernel_spmd(nc, per_core, core_ids=list(range(NCORES)))
    out = np.zeros((B, N, 3), np.float32)
    for b in range(B):
        out[b, perm] = res.results[b]["out_t"].T
    return out


if __name__ == "__main__":
    rng = np.random.default_rng(0)
    # lightweight self-test with random inputs
    inputs = {
        "feature_grid": rng.standard_normal((B, H, W, D), dtype=np.float32),
        "context_vector": rng.standard_normal((B, D), dtype=np.float32),
        "coords": rng.uniform(-1, 1, (N, 2)).astype(np.float32),
        "oracle_pixels": rng.uniform(0, 1, (B, N, 3)).astype(np.float32),
        "mlp0_w": (rng.standard_normal((813, 256)) / np.sqrt(813)).astype(np.float32),
        "mlp0_b": np.zeros(256, np.float32),
        "mlp_hw": (rng.standard_normal((3, 256, 256)) / 16).astype(np.float32),
        "mlp_hb": np.zeros((3, 256), np.float32),
        "ctx_w": (rng.standard_normal((256, 512)) / 16).astype(np.float32),
        "ctx_b": np.zeros(512, np.float32),
        "out_w": (rng.standard_normal((256, 3)) / 16 * 0.01).astype(np.float32),
        "out_b": np.zeros(3, np.float32),
    }
    out = kernel(**inputs)
    print("kernel out:", out.shape, out.dtype, np.abs(out).max())
